# revision 25
# baseline (speedup 1.0000x reference)
"""Trainium2 Bass kernel for autoregressive multi-head self-attention.

Problem: B=2, S=2048, H=2048 (16 heads x 128), RoPE, causal softmax with the
(faithful-to-source) sqrt(head_dim) score MULTIPLIER, out projection.

Sharding: 8 cores = 2 (batch) x 4 (head-groups of 4 heads). Attention is fully
local per core. Out-proj is row-parallel: each core emits two partial [S, H]
bf16 outputs (one per head-pair pass); host sums the 8 partials per batch
element.

All matmuls run as float32r (fp32 storage, reduced-precision PE fast path,
1 cycle/row at N>=256). Softmax is exact fp32 on DVE/ACT. The out-projection
is interleaved into the attention passes (one supertile behind) so its
matmuls fill PE gaps left by the softmax DVE/ACT chain.
"""

import math
import sys

sys.path.insert(0, "/opt/trn_rl_repo")

import ml_dtypes
import numpy as np

import concourse.bacc as bacc
import concourse.tile as tile
from concourse import bass_utils, mybir
from contextlib import ExitStack

P = 128          # partitions / head dim / q,k,v tile
S = 2048         # sequence length
H = 2048         # hidden
NH = 16          # total heads
HPC = 4          # heads per core
NCORES = 8
SC = 512         # s-chunk width for projections
NCT = H // P     # 16 c-tiles (contraction)
NQT = S // P     # 16 q tiles
NEG = -1.0e30

R32 = mybir.dt.float32r
F32 = mybir.dt.float32
BF16 = mybir.dt.bfloat16
AX = mybir.AxisListType.X
EXP = mybir.ActivationFunctionType.Exp

CFG = {"xb": 5, "probs": 2, "pta": 2, "qt": 2, "m": 2, "sch": 512,
       "psqk": 2, "psc": 2, "rpv": 1, "pcx": 1, "po": 2, "ost": 1}


def _build_program(loop_iters=None):
    nc = bacc.Bacc("TRN2", target_bir_lowering=False, debug=False)

    xT = nc.dram_tensor("xT", [H, S], R32, kind="ExternalInput")        # x[b].T
    wqT = nc.dram_tensor("wqT", [H, HPC * P], R32, kind="ExternalInput")
    wkT = nc.dram_tensor("wkT", [H, HPC * P], R32, kind="ExternalInput")
    wvT = nc.dram_tensor("wvT", [H, HPC * P], R32, kind="ExternalInput")
    woT = nc.dram_tensor("woT", [HPC * P, H], BF16, kind="ExternalInput")
    cosq = nc.dram_tensor("cosq", [P, S], R32, kind="ExternalInput")    # * sqrt(hd)
    sinq = nc.dram_tensor("sinq", [P, S], R32, kind="ExternalInput")    # * sqrt(hd)
    cosk = nc.dram_tensor("cosk", [P, S], R32, kind="ExternalInput")
    sink = nc.dram_tensor("sink", [P, S], R32, kind="ExternalInput")
    permT = nc.dram_tensor("permT", [P, P], R32, kind="ExternalInput")  # rot-half
    maskc = nc.dram_tensor("maskc", [P, P], R32, kind="ExternalInput")  # causal add
    ident = nc.dram_tensor("ident", [P, P], R32, kind="ExternalInput")
    out = nc.dram_tensor("out", [S, H], BF16, kind="ExternalOutput")

    with tile.TileContext(nc) as tc, ExitStack() as ctx:
        if loop_iters is not None:
            ctx.enter_context(tc.For_i(0, loop_iters, 1))
        cpool = ctx.enter_context(tc.tile_pool(name="consts", bufs=1))
        mask_sb = cpool.tile([P, P], R32, tag="mask", name="mask_sb")
        perm_sb = cpool.tile([P, P], R32, tag="perm", name="perm_sb")
        id_sb = cpool.tile([P, P], R32, tag="ident", name="id_sb")
        nc.gpsimd.dma_start(out=mask_sb, in_=maskc.ap())
        nc.gpsimd.dma_start(out=perm_sb, in_=permT.ap())
        nc.gpsimd.dma_start(out=id_sb, in_=ident.ap())
        ctxpool = ctx.enter_context(tc.tile_pool(name="ctxp", bufs=1))
        ctxTall = [
            ctxpool.tile([P, S], BF16, tag=f"ctxT{h}", name=f"ctxT{h}")
            for h in range(HPC)
        ]

        for hp in range(2):  # head-pair passes: heads {2hp, 2hp+1}
            with ExitStack() as pctx:
                wpool = pctx.enter_context(tc.tile_pool(name=f"w{hp}", bufs=1))
                wq_sb = wpool.tile([P, NCT, 2 * P], R32, tag="wq", name=f"wq{hp}")
                wk_sb = wpool.tile([P, NCT, 2 * P], R32, tag="wk", name=f"wk{hp}")
                wv_sb = wpool.tile([P, NCT, 2 * P], R32, tag="wv", name=f"wv{hp}")
                osl = slice(hp * 2 * P, (hp + 1) * 2 * P)
                def load_w(wsb, wdr):
                    for wg in range(4):
                        nc.sync.dma_start(
                            out=wsb[:, wg * 4 : (wg + 1) * 4, :],
                            in_=wdr.ap()[wg * 4 * P : (wg + 1) * 4 * P, osl]
                            .rearrange("(t p) o -> p t o", p=P),
                        )
                # q/k weights now; v and out-proj weights deferred until
                # after chunk 0's x loads are queued (same HWDGE ring)
                load_w(wq_sb, wqT)
                load_w(wk_sb, wkT)
                wo_sb = (
                    wpool.tile([P, HPC, H], BF16, tag="wo", name="wo_sb")
                    if hp == 1 else None
                )

                kvpool = pctx.enter_context(tc.tile_pool(name=f"kv{hp}", bufs=1))
                kT = [
                    kvpool.tile([P, S], R32, tag=f"kT{i}", name=f"kT{hp}_{i}")
                    for i in range(2)
                ]
                v_sb = kvpool.tile([P, NQT, 2 * P], BF16, tag="v", name=f"v{hp}")
                ctxT = ctxTall[hp * 2 : hp * 2 + 2]

                # merged projection + attention pools
                xpool = pctx.enter_context(tc.tile_pool(name=f"x{hp}", bufs=CFG["xb"]))
                tpool = pctx.enter_context(tc.tile_pool(name=f"t{hp}", bufs=4))
                mpool = pctx.enter_context(tc.tile_pool(name=f"m{hp}", bufs=CFG["m"]))
                qtpool = pctx.enter_context(tc.tile_pool(name=f"qt{hp}", bufs=CFG["qt"]))
                ppool = pctx.enter_context(tc.tile_pool(name=f"pr{hp}", bufs=CFG["probs"]))
                ptapool = pctx.enter_context(tc.tile_pool(name=f"pt{hp}", bufs=CFG["pta"]))
                smpool = pctx.enter_context(tc.tile_pool(name=f"sm{hp}", bufs=4))
                ostpool = pctx.enter_context(tc.tile_pool(name=f"os{hp}", bufs=CFG["ost"]))
                psqk_pool = pctx.enter_context(
                    tc.tile_pool(name=f"pqk{hp}", bufs=CFG["psqk"], space="PSUM")
                )
                rotpv_pool = pctx.enter_context(
                    tc.tile_pool(name=f"rpv{hp}", bufs=CFG["rpv"], space="PSUM")
                )
                psc_pool = pctx.enter_context(
                    tc.tile_pool(name=f"psc{hp}", bufs=CFG["psc"], space="PSUM")
                )
                pcx_pool = pctx.enter_context(
                    tc.tile_pool(name=f"pcx{hp}", bufs=CFG["pcx"], space="PSUM")
                )
                po_pool = pctx.enter_context(
                    tc.tile_pool(name=f"po{hp}", bufs=CFG["po"], space="PSUM")
                )

                def rope(dest, ps, cos_t, sin_t, nm):
                    raw = mpool.tile([P, SC], R32, tag="qraw", name=f"raw{nm}")
                    nc.scalar.copy(out=raw, in_=ps)
                    rot = rotpv_pool.tile(
                        [P, SC], F32, tag="rotpv", name=f"rot{nm}"
                    )
                    nc.tensor.matmul(
                        rot, lhsT=perm_sb, rhs=raw, start=True, stop=True
                    )
                    nc.vector.tensor_mul(out=dest, in0=ps, in1=cos_t)
                    tmp = mpool.tile([P, SC], R32, tag="rtmp", name=f"tmp{nm}")
                    nc.vector.tensor_mul(out=tmp, in0=rot, in1=sin_t)
                    nc.vector.tensor_add(out=dest, in0=dest, in1=tmp)

                def outproj(Q):
                    """Row-parallel out-proj for supertile Q's 4 s-tiles
                    over all 4 heads (runs in pass 1 only)."""
                    for st4 in range(4):
                        st = Q * 4 + st4
                        ost = ostpool.tile([P, H], BF16, tag="ost",
                                           name=f"ost{hp}_{st}")
                        for oc in range(4):
                            po = po_pool.tile([P, 512], F32, tag="po",
                                              name=f"po{hp}_{st}_{oc}")
                            for h4 in range(HPC):
                                nc.tensor.matmul(
                                    po,
                                    lhsT=ctxTall[h4][:, st * P : (st + 1) * P],
                                    rhs=wo_sb[:, h4, oc * 512 : (oc + 1) * 512],
                                    start=(h4 == 0),
                                    stop=(h4 == HPC - 1),
                                )
                            nc.any.tensor_copy(
                                out=ost[:, oc * 512 : (oc + 1) * 512], in_=po
                            )
                        nc.sync.dma_start(
                            out=out.ap()[st * P : (st + 1) * P, :],
                            in_=ost,
                        )

                for sc in range(S // SC):
                    ssl = slice(sc * SC, (sc + 1) * SC)
                    # ---- project s-chunk sc ----
                    xg = []
                    for g in range(4):  # 4 c-tiles per SBUF tile
                        t = xpool.tile(
                            [P, 4, SC], R32, tag="xb", name=f"xb{hp}_{sc}_{g}"
                        )
                        nc.sync.dma_start(
                            out=t,
                            in_=xT.ap()[g * 4 * P : (g + 1) * 4 * P, ssl]
                            .rearrange("(t p) s -> p t s", p=P),
                        )
                        xg.append(t)
                    xb = [xg[ct // 4][:, ct % 4, :] for ct in range(NCT)]
                    if sc == 0:
                        # deferred: behind chunk-0 x loads on the HWDGE ring
                        load_w(wv_sb, wvT)
                        if hp == 1:
                            nc.sync.dma_start(
                                out=wo_sb,
                                in_=woT.ap().rearrange("(t p) o -> p t o", p=P),
                            )
                    trig = {}
                    for tn, tdr in (
                        ("cq", cosq), ("sq", sinq), ("ck", cosk), ("sk", sink)
                    ):
                        t = tpool.tile(
                            [P, SC], R32, tag="trig", name=f"{tn}{hp}_{sc}"
                        )
                        nc.gpsimd.dma_start(out=t, in_=tdr.ap()[:, ssl])
                        trig[tn] = t

                    qts = []
                    for hh in range(2):
                        hsl = slice(hh * P, (hh + 1) * P)
                        psq = psqk_pool.tile(
                            [P, SC], F32, tag="pqk", name=f"psq{hp}{sc}{hh}"
                        )
                        for ct in range(NCT):
                            nc.tensor.matmul(
                                psq,
                                lhsT=wq_sb[:, ct, hsl],
                                rhs=xb[ct],
                                start=(ct == 0),
                                stop=(ct == NCT - 1),
                            )
                        qt = qtpool.tile(
                            [P, SC], R32, tag=f"qt{hh}", name=f"qt{hp}{sc}{hh}"
                        )
                        qts.append(qt)
                        rope(qt, psq, trig["cq"], trig["sq"],
                             f"q{hp}{sc}{hh}")
                        psk = psqk_pool.tile(
                            [P, SC], F32, tag="pqk", name=f"psk{hp}{sc}{hh}"
                        )
                        for ct in range(NCT):
                            nc.tensor.matmul(
                                psk,
                                lhsT=wk_sb[:, ct, hsl],
                                rhs=xb[ct],
                                start=(ct == 0),
                                stop=(ct == NCT - 1),
                            )
                        rope(kT[hh][:, ssl], psk, trig["ck"], trig["sk"],
                             f"k{hp}{sc}{hh}")

                    for sti in range(SC // P):
                        st = sc * (SC // P) + sti
                        psv = rotpv_pool.tile(
                            [P, SC], F32, tag="rotpv", name=f"pv{hp}{st}"
                        )
                        for ct in range(NCT):
                            nc.tensor.matmul(
                                psv[:, : 2 * P],
                                lhsT=xb[ct][:, sti * P : (sti + 1) * P],
                                rhs=wv_sb[:, ct, :],
                                start=(ct == 0),
                                stop=(ct == NCT - 1),
                            )
                        nc.any.tensor_copy(out=v_sb[:, st, :], in_=psv[:, : 2 * P])

                    # ---- attention supertile Q = sc for both heads ----
                    Q = sc
                    for hh in range(2):
                        h = hp * 2 + hh
                        ptq2 = None
                        pt2 = None
                        nkt2 = None
                        for qi_in in range(4):
                            qi = Q * 4 + qi_in
                            L = (qi + 1) * P
                            CH = CFG["sch"]
                            nch = (L + CH - 1) // CH
                            if qi_in % 2 == 0:
                                # probsT for this qi-pair, exact extent so the
                                # pair transpose output is contiguous:
                                # [k, qcol, kt, q] with kt = qi1+1 tiles
                                nkt2 = qi + 2
                                ptq2 = ptapool.tile(
                                    [P, 2, nkt2, P], BF16, tag="pta",
                                    name=f"ptq{h}{qi}",
                                )
                                # flat probs tile for the pair; qi0 at base 0
                                # (L1 wide, gap zeroed), qi1 at base L1
                                pt2 = ppool.tile(
                                    [P, 2 * S], BF16, tag="probs",
                                    name=f"pr{h}{qi}",
                                )
                                # zero qi0's causal-overhang gap [L0:L1]
                                nc.any.memset(pt2[:, L : L + P], 0.0)
                            base = 0 if qi_in % 2 == 0 else L
                            pt = pt2[:, base : base + L]
                            # online (per-chunk) softmax: each chunk is
                            # exp'd against its own max immediately (frees
                            # the PSUM bank), then rescaled at the end.
                            maxn = smpool.tile(
                                [P, 4], F32, tag="maxp", name=f"mx{h}{qi}"
                            )
                            sums = smpool.tile(
                                [P, 4], F32, tag="sums", name=f"sm{h}{qi}"
                            )
                            chunks = []
                            for cn in range(nch):
                                n0 = cn * CH
                                w = min(L, n0 + CH) - n0
                                chunks.append((n0, w))
                                psc = psc_pool.tile(
                                    [P, CFG["sch"]], F32, tag="sc", name=f"sc{h}{qi}{cn}"
                                )
                                # diagonal block lives in this chunk?
                                has_diag = n0 <= qi * P < n0 + w
                                nc.tensor.matmul(
                                    psc[:, :w],
                                    lhsT=qts[hh][
                                        :, qi_in * P : (qi_in + 1) * P
                                    ],
                                    rhs=kT[hh][:, n0 : n0 + w],
                                    start=True,
                                    stop=not has_diag,
                                )
                                if has_diag:
                                    # causal mask as PE accumulate:
                                    # psc_diag += I.T @ mask
                                    off = qi * P - n0
                                    nc.tensor.matmul(
                                        psc[:, off : off + P],
                                        lhsT=id_sb,
                                        rhs=mask_sb,
                                        start=False,
                                        stop=True,
                                    )
                                nc.vector.reduce_max(
                                    out=maxn[:, cn : cn + 1],
                                    in_=psc[:, :w],
                                    axis=AX,
                                    negate=True,
                                )
                                nc.scalar.activation(
                                    out=pt[:, n0 : n0 + w],
                                    in_=psc[:, :w],
                                    func=EXP,
                                    bias=maxn[:, cn : cn + 1],
                                    scale=1.0,
                                    accum_out=sums[:, cn : cn + 1],
                                )
                            recip = smpool.tile(
                                [P, 1], F32, tag="recip", name=f"rc{h}{qi}"
                            )
                            if nch == 1:
                                nc.vector.reciprocal(out=recip, in_=sums[:, 0:1])
                                nc.vector.tensor_scalar_mul(
                                    pt[:, :L], pt[:, :L], recip
                                )
                            else:
                                rowneg = smpool.tile(
                                    [P, 1], F32, tag="rneg", name=f"rn{h}{qi}"
                                )
                                # maxn holds -m_c; row -m = min_c(-m_c)
                                nc.vector.tensor_reduce(
                                    out=rowneg, in_=maxn[:, :nch], axis=AX,
                                    op=mybir.AluOpType.min,
                                )
                                fc = smpool.tile(
                                    [P, 4], F32, tag="fc", name=f"fc{h}{qi}"
                                )
                                # f_c = exp(m_c - m) = exp(-1*maxn + rowneg)
                                nc.scalar.activation(
                                    out=fc[:, :nch],
                                    in_=maxn[:, :nch],
                                    func=EXP,
                                    bias=rowneg,
                                    scale=-1.0,
                                )
                                sums2 = smpool.tile(
                                    [P, 4], F32, tag="sums2", name=f"s2{h}{qi}"
                                )
                                nc.vector.tensor_mul(
                                    out=sums2[:, :nch], in0=sums[:, :nch],
                                    in1=fc[:, :nch],
                                )
                                ssum = smpool.tile(
                                    [P, 1], F32, tag="ssum", name=f"ss{h}{qi}"
                                )
                                nc.vector.reduce_sum(
                                    out=ssum, in_=sums2[:, :nch], axis=AX
                                )
                                nc.vector.reciprocal(out=recip, in_=ssum)
                                scl = smpool.tile(
                                    [P, 4], F32, tag="scl", name=f"sl{h}{qi}"
                                )
                                nc.vector.tensor_scalar_mul(
                                    scl[:, :nch], fc[:, :nch], recip
                                )
                                for cn, (n0, w) in enumerate(chunks):
                                    nc.vector.tensor_scalar_mul(
                                        pt[:, n0 : n0 + w], pt[:, n0 : n0 + w],
                                        scl[:, cn : cn + 1],
                                    )

                            if qi_in % 2 == 1:
                                # one xbar transpose for the whole pair:
                                # [q, 2*L1] -> [k, qcol, kt, q]
                                nc.scalar.dma_start(
                                    out=ptq2,
                                    in_=pt2[:, : 2 * L],
                                    transpose=True,
                                )
                                # PV for this qi-pair (N=256 per k-tile)
                                ctps = pcx_pool.tile(
                                    [P, 2 * P], F32, tag="cx",
                                    name=f"cx{h}{qi}",
                                )
                                for kt in range(qi + 1):
                                    nc.tensor.matmul(
                                        ctps,
                                        lhsT=v_sb[:, kt, hh * P : (hh + 1) * P],
                                        rhs=ptq2[:, :, kt, :],
                                        start=(kt == 0),
                                        stop=(kt == qi),
                                    )
                                nc.scalar.copy(
                                    out=ctxT[hh][
                                        :, (qi - 1) * P : (qi + 1) * P
                                    ],
                                    in_=ctps,
                                )

                    # ---- out-proj for the previous supertile (PE gap filler)
                    if hp == 1 and sc > 0:
                        outproj(sc - 1)

                if hp == 1:
                    outproj(3)

    nc.compile()
    return nc


_NC_CACHE = None


def _get_program():
    global _NC_CACHE
    if _NC_CACHE is None:
        _NC_CACHE = _build_program()
    return _NC_CACHE


def _host_inputs(x, Wq, Wk, Wv, Wo, cos, sin):
    """Build the 8 per-core input maps (host-side sharding + layout prep)."""
    B = x.shape[0]
    sq = math.sqrt(P)

    cosT = np.ascontiguousarray(cos[:S].T.astype(np.float32))  # [128, S]
    sinT = np.ascontiguousarray(sin[:S].T.astype(np.float32))

    # rotate-half as a signed permutation: rot[d] = sign(d) * x[(d+64) % 128]
    perm = np.zeros((P, P), np.float32)
    for d in range(P):
        perm[d, (d + P // 2) % P] = -1.0 if d < P // 2 else 1.0
    permT_np = np.ascontiguousarray(perm.T)

    mask_np = np.triu(np.full((P, P), NEG, np.float32), k=1)
    ident_np = np.eye(P, dtype=np.float32)

    xTb = [np.ascontiguousarray(x[b].T.astype(np.float32)) for b in range(B)]

    in_maps = []
    for core in range(NCORES):
        b = core // 4
        hg = core % 4
        rows = slice(hg * HPC * P, (hg + 1) * HPC * P)
        in_maps.append(
            {
                "xT": xTb[b],
                "wqT": np.ascontiguousarray(Wq[rows, :].T.astype(np.float32)),
                "wkT": np.ascontiguousarray(Wk[rows, :].T.astype(np.float32)),
                "wvT": np.ascontiguousarray(Wv[rows, :].T.astype(np.float32)),
                "woT": np.ascontiguousarray(Wo[:, rows].T.astype(ml_dtypes.bfloat16)),
                "cosq": np.ascontiguousarray(cosT * sq),
                "sinq": np.ascontiguousarray(sinT * sq),
                "cosk": cosT,
                "sink": sinT,
                "permT": permT_np,
                "maskc": mask_np,
                "ident": ident_np,
            }
        )
    return in_maps


def kernel(x, Wq, Wk, Wv, Wo, cos, sin, _trace=False):
    x, Wq, Wk, Wv, Wo, cos, sin = (
        np.asarray(a, dtype=np.float32) for a in (x, Wq, Wk, Wv, Wo, cos, sin)
    )
    nc = _get_program()
    in_maps = _host_inputs(x, Wq, Wk, Wv, Wo, cos, sin)
    res = bass_utils.run_bass_kernel_spmd(
        nc, in_maps, core_ids=list(range(NCORES)), trace=_trace
    )
    kernel.last_result = res
    B = x.shape[0]
    out = np.zeros((B, S, H), np.float32)
    for core in range(NCORES):
        r = res.results[core]
        out[core // 4] += r["out"].astype(np.float32)
    return out


# revision 26
# speedup vs baseline: 1.0919x; 1.0919x over previous
"""Trainium2 Bass kernel for autoregressive multi-head self-attention.

Problem: B=2, S=2048, H=2048 (16 heads x 128), RoPE, causal softmax with the
(faithful-to-source) sqrt(head_dim) score MULTIPLIER, out projection.

Sharding: 8 cores = 2 (batch) x 4 (head-groups of 4 heads). Attention is fully
local per core. Out-proj is row-parallel: each core emits two partial [S, H]
bf16 outputs (one per head-pair pass); host sums the 8 partials per batch
element.

All matmuls run as float32r (fp32 storage, reduced-precision PE fast path,
1 cycle/row at N>=256). Softmax is exact fp32 on DVE/ACT. The out-projection
is interleaved into the attention passes (one supertile behind) so its
matmuls fill PE gaps left by the softmax DVE/ACT chain.
"""

import math
import sys

sys.path.insert(0, "/opt/trn_rl_repo")

import ml_dtypes
import numpy as np

import concourse.bacc as bacc
import concourse.tile as tile
from concourse import bass_utils, mybir
from contextlib import ExitStack

P = 128          # partitions / head dim / q,k,v tile
S = 2048         # sequence length
H = 2048         # hidden
NH = 16          # total heads
HPC = 4          # heads per core
NCORES = 8
SC = 512         # s-chunk width for projections
NCT = H // P     # 16 c-tiles (contraction)
NQT = S // P     # 16 q tiles
NEG = -1.0e30

R32 = mybir.dt.float32r
F32 = mybir.dt.float32
BF16 = mybir.dt.bfloat16
AX = mybir.AxisListType.X
EXP = mybir.ActivationFunctionType.Exp

CFG = {"xb": 5, "probs": 2, "pta": 2, "qt": 2, "m": 2, "sch": 512,
       "psqk": 2, "psc": 2, "rpv": 1, "pcx": 1, "po": 2, "ost": 2}


def _build_program(loop_iters=None):
    nc = bacc.Bacc("TRN2", target_bir_lowering=False, debug=False)

    xT = nc.dram_tensor("xT", [H, S], R32, kind="ExternalInput")        # x[b].T
    wqT = nc.dram_tensor("wqT", [H, HPC * P], R32, kind="ExternalInput")
    wkT = nc.dram_tensor("wkT", [H, HPC * P], R32, kind="ExternalInput")
    wvT = nc.dram_tensor("wvT", [H, HPC * P], R32, kind="ExternalInput")
    woT = nc.dram_tensor("woT", [HPC * P, H], BF16, kind="ExternalInput")
    cosq = nc.dram_tensor("cosq", [P, S], R32, kind="ExternalInput")    # * sqrt(hd)
    sinq = nc.dram_tensor("sinq", [P, S], R32, kind="ExternalInput")    # * sqrt(hd)
    cosk = nc.dram_tensor("cosk", [P, S], R32, kind="ExternalInput")
    sink = nc.dram_tensor("sink", [P, S], R32, kind="ExternalInput")
    permT = nc.dram_tensor("permT", [P, P], R32, kind="ExternalInput")  # rot-half
    maskc = nc.dram_tensor("maskc", [P, P], R32, kind="ExternalInput")  # causal add
    ident = nc.dram_tensor("ident", [P, P], R32, kind="ExternalInput")
    out = nc.dram_tensor("out", [S, H], BF16, kind="ExternalOutput")

    with tile.TileContext(nc) as tc, ExitStack() as ctx:
        if loop_iters is not None:
            ctx.enter_context(tc.For_i(0, loop_iters, 1))
        cpool = ctx.enter_context(tc.tile_pool(name="consts", bufs=1))
        mask_sb = cpool.tile([P, P], R32, tag="mask", name="mask_sb")
        perm_sb = cpool.tile([P, P], R32, tag="perm", name="perm_sb")
        id_sb = cpool.tile([P, P], R32, tag="ident", name="id_sb")
        nc.gpsimd.dma_start(out=mask_sb, in_=maskc.ap())
        nc.gpsimd.dma_start(out=perm_sb, in_=permT.ap())
        nc.gpsimd.dma_start(out=id_sb, in_=ident.ap())
        ctxpool = ctx.enter_context(tc.tile_pool(name="ctxp", bufs=1))
        ctxTall = [
            ctxpool.tile([P, S], BF16, tag=f"ctxT{h}", name=f"ctxT{h}")
            for h in range(HPC)
        ]

        for hp in range(2):  # head-pair passes: heads {2hp, 2hp+1}
            with ExitStack() as pctx:
                wpool = pctx.enter_context(tc.tile_pool(name=f"w{hp}", bufs=1))
                wq_sb = wpool.tile([P, NCT, 2 * P], R32, tag="wq", name=f"wq{hp}")
                wk_sb = wpool.tile([P, NCT, 2 * P], R32, tag="wk", name=f"wk{hp}")
                wv_sb = wpool.tile([P, NCT, 2 * P], R32, tag="wv", name=f"wv{hp}")
                osl = slice(hp * 2 * P, (hp + 1) * 2 * P)
                def load_w(wsb, wdr):
                    for wg in range(4):
                        nc.sync.dma_start(
                            out=wsb[:, wg * 4 : (wg + 1) * 4, :],
                            in_=wdr.ap()[wg * 4 * P : (wg + 1) * 4 * P, osl]
                            .rearrange("(t p) o -> p t o", p=P),
                        )
                # q/k weights now; v and out-proj weights deferred until
                # after chunk 0's x loads are queued (same HWDGE ring)
                load_w(wq_sb, wqT)
                load_w(wk_sb, wkT)
                wo_sb = (
                    wpool.tile([P, HPC, H], BF16, tag="wo", name="wo_sb")
                    if hp == 1 else None
                )

                kvpool = pctx.enter_context(tc.tile_pool(name=f"kv{hp}", bufs=1))
                kT = [
                    kvpool.tile([P, S], R32, tag=f"kT{i}", name=f"kT{hp}_{i}")
                    for i in range(2)
                ]
                v_sb = kvpool.tile([P, NQT, 2 * P], BF16, tag="v", name=f"v{hp}")
                ctxT = ctxTall[hp * 2 : hp * 2 + 2]

                # merged projection + attention pools
                xpool = pctx.enter_context(tc.tile_pool(name=f"x{hp}", bufs=CFG["xb"]))
                tpool = pctx.enter_context(tc.tile_pool(name=f"t{hp}", bufs=4))
                mpool = pctx.enter_context(tc.tile_pool(name=f"m{hp}", bufs=CFG["m"]))
                qtpool = pctx.enter_context(tc.tile_pool(name=f"qt{hp}", bufs=CFG["qt"]))
                ppool = pctx.enter_context(tc.tile_pool(name=f"pr{hp}", bufs=CFG["probs"]))
                ptapool = pctx.enter_context(tc.tile_pool(name=f"pt{hp}", bufs=CFG["pta"]))
                smpool = pctx.enter_context(tc.tile_pool(name=f"sm{hp}", bufs=4))
                ostpool = pctx.enter_context(tc.tile_pool(name=f"os{hp}", bufs=CFG["ost"]))
                psqk_pool = pctx.enter_context(
                    tc.tile_pool(name=f"pqk{hp}", bufs=CFG["psqk"], space="PSUM")
                )
                rotpv_pool = pctx.enter_context(
                    tc.tile_pool(name=f"rpv{hp}", bufs=CFG["rpv"], space="PSUM")
                )
                psc_pool = pctx.enter_context(
                    tc.tile_pool(name=f"psc{hp}", bufs=CFG["psc"], space="PSUM")
                )
                pcx_pool = pctx.enter_context(
                    tc.tile_pool(name=f"pcx{hp}", bufs=CFG["pcx"], space="PSUM")
                )
                po_pool = pctx.enter_context(
                    tc.tile_pool(name=f"po{hp}", bufs=CFG["po"], space="PSUM")
                )

                def rope(dest, ps, cos_t, sin_t, nm):
                    raw = mpool.tile([P, SC], R32, tag="qraw", name=f"raw{nm}")
                    nc.scalar.copy(out=raw, in_=ps)
                    rot = rotpv_pool.tile(
                        [P, SC], F32, tag="rotpv", name=f"rot{nm}"
                    )
                    nc.tensor.matmul(
                        rot, lhsT=perm_sb, rhs=raw, start=True, stop=True
                    )
                    nc.vector.tensor_mul(out=dest, in0=ps, in1=cos_t)
                    tmp = mpool.tile([P, SC], R32, tag="rtmp", name=f"tmp{nm}")
                    nc.vector.tensor_mul(out=tmp, in0=rot, in1=sin_t)
                    nc.vector.tensor_add(out=dest, in0=dest, in1=tmp)

                def outproj(Q):
                    """Row-parallel out-proj for supertile Q's 4 s-tiles
                    over all 4 heads (runs in pass 1 only)."""
                    for st4 in range(4):
                        st = Q * 4 + st4
                        ost = ostpool.tile([P, H], BF16, tag="ost",
                                           name=f"ost{hp}_{st}")
                        for oc in range(4):
                            po = po_pool.tile([P, 512], F32, tag="po",
                                              name=f"po{hp}_{st}_{oc}")
                            for h4 in range(HPC):
                                nc.tensor.matmul(
                                    po,
                                    lhsT=ctxTall[h4][:, st * P : (st + 1) * P],
                                    rhs=wo_sb[:, h4, oc * 512 : (oc + 1) * 512],
                                    start=(h4 == 0),
                                    stop=(h4 == HPC - 1),
                                )
                            nc.any.tensor_copy(
                                out=ost[:, oc * 512 : (oc + 1) * 512], in_=po
                            )
                        nc.sync.dma_start(
                            out=out.ap()[st * P : (st + 1) * P, :],
                            in_=ost,
                        )

                for sc in range(S // SC):
                    ssl = slice(sc * SC, (sc + 1) * SC)
                    # ---- project s-chunk sc ----
                    xg = []
                    for g in range(4):  # 4 c-tiles per SBUF tile
                        t = xpool.tile(
                            [P, 4, SC], R32, tag="xb", name=f"xb{hp}_{sc}_{g}"
                        )
                        nc.sync.dma_start(
                            out=t,
                            in_=xT.ap()[g * 4 * P : (g + 1) * 4 * P, ssl]
                            .rearrange("(t p) s -> p t s", p=P),
                        )
                        xg.append(t)
                    xb = [xg[ct // 4][:, ct % 4, :] for ct in range(NCT)]
                    if sc == 0:
                        # deferred: behind chunk-0 x loads on the HWDGE ring
                        load_w(wv_sb, wvT)
                        if hp == 1:
                            nc.sync.dma_start(
                                out=wo_sb,
                                in_=woT.ap().rearrange("(t p) o -> p t o", p=P),
                            )
                    trig = {}
                    for tn, tdr in (
                        ("cq", cosq), ("sq", sinq), ("ck", cosk), ("sk", sink)
                    ):
                        t = tpool.tile(
                            [P, SC], R32, tag="trig", name=f"{tn}{hp}_{sc}"
                        )
                        nc.gpsimd.dma_start(out=t, in_=tdr.ap()[:, ssl])
                        trig[tn] = t

                    qts = []
                    for hh in range(2):
                        hsl = slice(hh * P, (hh + 1) * P)
                        psq = psqk_pool.tile(
                            [P, SC], F32, tag="pqk", name=f"psq{hp}{sc}{hh}"
                        )
                        for ct in range(NCT):
                            nc.tensor.matmul(
                                psq,
                                lhsT=wq_sb[:, ct, hsl],
                                rhs=xb[ct],
                                start=(ct == 0),
                                stop=(ct == NCT - 1),
                            )
                        qt = qtpool.tile(
                            [P, SC], R32, tag=f"qt{hh}", name=f"qt{hp}{sc}{hh}"
                        )
                        qts.append(qt)
                        rope(qt, psq, trig["cq"], trig["sq"],
                             f"q{hp}{sc}{hh}")
                        psk = psqk_pool.tile(
                            [P, SC], F32, tag="pqk", name=f"psk{hp}{sc}{hh}"
                        )
                        for ct in range(NCT):
                            nc.tensor.matmul(
                                psk,
                                lhsT=wk_sb[:, ct, hsl],
                                rhs=xb[ct],
                                start=(ct == 0),
                                stop=(ct == NCT - 1),
                            )
                        rope(kT[hh][:, ssl], psk, trig["ck"], trig["sk"],
                             f"k{hp}{sc}{hh}")

                    for sti in range(SC // P):
                        st = sc * (SC // P) + sti
                        psv = rotpv_pool.tile(
                            [P, SC], F32, tag="rotpv", name=f"pv{hp}{st}"
                        )
                        for ct in range(NCT):
                            nc.tensor.matmul(
                                psv[:, : 2 * P],
                                lhsT=xb[ct][:, sti * P : (sti + 1) * P],
                                rhs=wv_sb[:, ct, :],
                                start=(ct == 0),
                                stop=(ct == NCT - 1),
                            )
                        nc.any.tensor_copy(out=v_sb[:, st, :], in_=psv[:, : 2 * P])

                    # ---- attention supertile Q = sc for both heads ----
                    Q = sc
                    for hh in range(2):
                        h = hp * 2 + hh
                        ptq2 = None
                        for qi_in in range(4):
                            qi = Q * 4 + qi_in
                            L = (qi + 1) * P
                            CH = CFG["sch"]
                            nch = (L + CH - 1) // CH
                            if qi_in % 2 == 0:
                                # probsT tile for this qi-pair: [k, kt, 2, q]
                                ptq2 = ptapool.tile(
                                    [P, NQT, 2, P], BF16, tag="pta",
                                    name=f"ptq{h}{qi}",
                                )
                                # zero the even-qi causal overhang at the
                                # odd qi's diagonal k-tile
                                nc.any.memset(ptq2[:, qi + 1, 0, :], 0.0)
                            pt = ppool.tile(
                                [P, S], BF16, tag="probs", name=f"pr{h}{qi}"
                            )
                            # online (per-chunk) softmax: each chunk is
                            # exp'd against its own max immediately (frees
                            # the PSUM bank), then rescaled at the end.
                            maxn = smpool.tile(
                                [P, 4], F32, tag="maxp", name=f"mx{h}{qi}"
                            )
                            sums = smpool.tile(
                                [P, 4], F32, tag="sums", name=f"sm{h}{qi}"
                            )
                            chunks = []
                            for cn in range(nch):
                                n0 = cn * CH
                                w = min(L, n0 + CH) - n0
                                chunks.append((n0, w))
                                psc = psc_pool.tile(
                                    [P, CFG["sch"]], F32, tag="sc", name=f"sc{h}{qi}{cn}"
                                )
                                # diagonal block lives in this chunk?
                                has_diag = n0 <= qi * P < n0 + w
                                nc.tensor.matmul(
                                    psc[:, :w],
                                    lhsT=qts[hh][
                                        :, qi_in * P : (qi_in + 1) * P
                                    ],
                                    rhs=kT[hh][:, n0 : n0 + w],
                                    start=True,
                                    stop=not has_diag,
                                )
                                if has_diag:
                                    # causal mask as PE accumulate:
                                    # psc_diag += I.T @ mask
                                    off = qi * P - n0
                                    nc.tensor.matmul(
                                        psc[:, off : off + P],
                                        lhsT=id_sb,
                                        rhs=mask_sb,
                                        start=False,
                                        stop=True,
                                    )
                                nc.vector.reduce_max(
                                    out=maxn[:, cn : cn + 1],
                                    in_=psc[:, :w],
                                    axis=AX,
                                    negate=True,
                                )
                                nc.scalar.activation(
                                    out=pt[:, n0 : n0 + w],
                                    in_=psc[:, :w],
                                    func=EXP,
                                    bias=maxn[:, cn : cn + 1],
                                    scale=1.0,
                                    accum_out=sums[:, cn : cn + 1],
                                )
                            recip = smpool.tile(
                                [P, 1], F32, tag="recip", name=f"rc{h}{qi}"
                            )
                            if nch == 1:
                                nc.vector.reciprocal(out=recip, in_=sums[:, 0:1])
                                nc.vector.tensor_scalar_mul(
                                    pt[:, :L], pt[:, :L], recip
                                )
                            else:
                                rowneg = smpool.tile(
                                    [P, 1], F32, tag="rneg", name=f"rn{h}{qi}"
                                )
                                # maxn holds -m_c; row -m = min_c(-m_c)
                                nc.vector.tensor_reduce(
                                    out=rowneg, in_=maxn[:, :nch], axis=AX,
                                    op=mybir.AluOpType.min,
                                )
                                fc = smpool.tile(
                                    [P, 4], F32, tag="fc", name=f"fc{h}{qi}"
                                )
                                # f_c = exp(m_c - m) = exp(-1*maxn + rowneg)
                                nc.scalar.activation(
                                    out=fc[:, :nch],
                                    in_=maxn[:, :nch],
                                    func=EXP,
                                    bias=rowneg,
                                    scale=-1.0,
                                )
                                sums2 = smpool.tile(
                                    [P, 4], F32, tag="sums2", name=f"s2{h}{qi}"
                                )
                                nc.vector.tensor_mul(
                                    out=sums2[:, :nch], in0=sums[:, :nch],
                                    in1=fc[:, :nch],
                                )
                                ssum = smpool.tile(
                                    [P, 1], F32, tag="ssum", name=f"ss{h}{qi}"
                                )
                                nc.vector.reduce_sum(
                                    out=ssum, in_=sums2[:, :nch], axis=AX
                                )
                                nc.vector.reciprocal(out=recip, in_=ssum)
                                scl = smpool.tile(
                                    [P, 4], F32, tag="scl", name=f"sl{h}{qi}"
                                )
                                nc.vector.tensor_scalar_mul(
                                    scl[:, :nch], fc[:, :nch], recip
                                )
                                for cn, (n0, w) in enumerate(chunks):
                                    nc.vector.tensor_scalar_mul(
                                        pt[:, n0 : n0 + w], pt[:, n0 : n0 + w],
                                        scl[:, cn : cn + 1],
                                    )

                            # one batched xbar transpose: [q, L] -> [k, kt, q]
                            nc.scalar.dma_start(
                                out=ptq2[:, : qi + 1, qi_in % 2, :],
                                in_=pt[:, :L],
                                transpose=True,
                            )

                            if qi_in % 2 == 1:
                                # PV for this qi-pair (N=256 per k-tile)
                                ctps = pcx_pool.tile(
                                    [P, 2 * P], F32, tag="cx",
                                    name=f"cx{h}{qi}",
                                )
                                for kt in range(qi + 1):
                                    nc.tensor.matmul(
                                        ctps,
                                        lhsT=v_sb[:, kt, hh * P : (hh + 1) * P],
                                        rhs=ptq2[:, kt, :, :],
                                        start=(kt == 0),
                                        stop=(kt == qi),
                                    )
                                nc.scalar.copy(
                                    out=ctxT[hh][
                                        :, (qi - 1) * P : (qi + 1) * P
                                    ],
                                    in_=ctps,
                                )

                    # ---- out-proj for the previous supertile (PE gap filler)
                    if hp == 1 and sc > 0:
                        outproj(sc - 1)

                if hp == 1:
                    outproj(3)

    nc.compile()
    return nc


_NC_CACHE = None


def _get_program():
    global _NC_CACHE
    if _NC_CACHE is None:
        _NC_CACHE = _build_program()
    return _NC_CACHE


def _host_inputs(x, Wq, Wk, Wv, Wo, cos, sin):
    """Build the 8 per-core input maps (host-side sharding + layout prep)."""
    B = x.shape[0]
    sq = math.sqrt(P)

    cosT = np.ascontiguousarray(cos[:S].T.astype(np.float32))  # [128, S]
    sinT = np.ascontiguousarray(sin[:S].T.astype(np.float32))

    # rotate-half as a signed permutation: rot[d] = sign(d) * x[(d+64) % 128]
    perm = np.zeros((P, P), np.float32)
    for d in range(P):
        perm[d, (d + P // 2) % P] = -1.0 if d < P // 2 else 1.0
    permT_np = np.ascontiguousarray(perm.T)

    mask_np = np.triu(np.full((P, P), NEG, np.float32), k=1)
    ident_np = np.eye(P, dtype=np.float32)

    xTb = [np.ascontiguousarray(x[b].T.astype(np.float32)) for b in range(B)]

    in_maps = []
    for core in range(NCORES):
        b = core // 4
        hg = core % 4
        rows = slice(hg * HPC * P, (hg + 1) * HPC * P)
        in_maps.append(
            {
                "xT": xTb[b],
                "wqT": np.ascontiguousarray(Wq[rows, :].T.astype(np.float32)),
                "wkT": np.ascontiguousarray(Wk[rows, :].T.astype(np.float32)),
                "wvT": np.ascontiguousarray(Wv[rows, :].T.astype(np.float32)),
                "woT": np.ascontiguousarray(Wo[:, rows].T.astype(ml_dtypes.bfloat16)),
                "cosq": np.ascontiguousarray(cosT * sq),
                "sinq": np.ascontiguousarray(sinT * sq),
                "cosk": cosT,
                "sink": sinT,
                "permT": permT_np,
                "maskc": mask_np,
                "ident": ident_np,
            }
        )
    return in_maps


def kernel(x, Wq, Wk, Wv, Wo, cos, sin, _trace=False):
    x, Wq, Wk, Wv, Wo, cos, sin = (
        np.asarray(a, dtype=np.float32) for a in (x, Wq, Wk, Wv, Wo, cos, sin)
    )
    nc = _get_program()
    in_maps = _host_inputs(x, Wq, Wk, Wv, Wo, cos, sin)
    res = bass_utils.run_bass_kernel_spmd(
        nc, in_maps, core_ids=list(range(NCORES)), trace=_trace
    )
    kernel.last_result = res
    B = x.shape[0]
    out = np.zeros((B, S, H), np.float32)
    for core in range(NCORES):
        r = res.results[core]
        out[core // 4] += r["out"].astype(np.float32)
    return out
